# revision 18
# baseline (speedup 1.0000x reference)
"""PEER / product-key MoE routing kernel for Trainium2 (8 NeuronCores).

Strategy: data-parallel over tokens. Each of the 8 cores gets 256 of the
2048 tokens plus a full replica of the expert tables in its DRAM. Routing
(q projection, product-key scores, two-stage top-8), expert-row gathers,
and the PEER combine all run on-device. No collectives are needed; the
host only slices/packs inputs and concatenates the per-core outputs.

Per-core pipeline (v2):
  PE:  qT = Wq^T @ x^T with full M=128 stationary tiles (fp32, exact),
       s1/s2 = q_half^T @ keys packed in disjoint partition halves
  DVE: top-8 via max8/max_index (exact fp32), 8x8 combo re-top-8,
       one-hot winner resolution
  GPSIMD: multi-index indirect-DMA gathers — each instruction fetches
       GS expert row-pairs per partition ([128, GS] int32 offsets into
       the host-packed [65536, 1024] bf16 wd|wu table)
  DVE+ACT: inner products as bf16 multiply (DVE 2x mode) + free-dim
       sum on the scalar engine (activation accum_out)
  PE:  combine as diag(vals) @ w_up rows accumulated across two PSUM
       banks per token block

Routing is fp32 end-to-end so expert selection matches the fp32
reference; tables are bf16 (rel err ~4e-3).
"""

import numpy as np

import concourse.bass as bass
import concourse.mybir as mybir
from concourse import bacc
from concourse.bass import IndirectOffsetOnAxis
from concourse.tile import TileContext
from concourse.bass_utils import run_bass_kernel_spmd

N_CORES = 8
N_HEADS = 8
D_KEYS = 128
HALF = 64
N_KEYS = 256
TOP_K = 8
D = 512
B = 2048           # total tokens
BC = B // N_CORES  # tokens per core (256)
TB = BC // 128     # token blocks per core (2)
GS = 8             # expert row-pairs gathered per partition per DMA
NG = 64 // GS      # gather groups per token block
F32 = mybir.dt.float32
U16 = mybir.dt.uint16
U32 = mybir.dt.uint32
I32 = mybir.dt.int32
BF16 = mybir.dt.bfloat16
X = mybir.AxisListType.X
OP = mybir.AluOpType
ACTF = mybir.ActivationFunctionType


def build_nc(stage="full"):
    nc = bacc.Bacc("TRN2", target_bir_lowering=False)

    xtokb_d = nc.dram_tensor("xtokb", [BC, D], BF16, kind="ExternalInput")
    xth_d = nc.dram_tensor("xth", [D, BC], BF16, kind="ExternalInput")
    xtl_d = nc.dram_tensor("xtl", [D, BC], BF16, kind="ExternalInput")
    wqh_d = nc.dram_tensor("wqh", [D, N_HEADS * D_KEYS], BF16,
                           kind="ExternalInput")
    wql_d = nc.dram_tensor("wql", [D, N_HEADS * D_KEYS], BF16,
                           kind="ExternalInput")
    bqp_d = nc.dram_tensor("bqp", [128, N_HEADS], F32, kind="ExternalInput")
    kpph_d = nc.dram_tensor("kpph", [128, N_HEADS, N_KEYS], BF16,
                            kind="ExternalInput")
    kppl_d = nc.dram_tensor("kppl", [128, N_HEADS, N_KEYS], BF16,
                            kind="ExternalInput")
    wb_d = nc.dram_tensor("wb", [N_KEYS * N_KEYS, 2 * D], BF16,
                          kind="ExternalInput")
    id01_d = nc.dram_tensor("id01", [128, 128], BF16, kind="ExternalInput")
    out_d = nc.dram_tensor("out", [BC, D], F32, kind="ExternalOutput")

    with TileContext(nc) as tc:
        with (
            tc.tile_pool(name="const", bufs=1) as cpool,
            tc.tile_pool(name="qt", bufs=1) as qtpool,
            tc.tile_pool(name="psq", bufs=1, space="PSUM") as psq,
            tc.tile_pool(name="pss", bufs=2, space="PSUM") as pss,
            tc.tile_pool(name="sc", bufs=4) as scpool,
            tc.tile_pool(name="st2", bufs=1) as st2,
            tc.tile_pool(name="eqs", bufs=2) as eqs,
            tc.tile_pool(name="pgp", bufs=24) as pgp,
            tc.tile_pool(name="ttr", bufs=2) as ttrp,
            tc.tile_pool(name="dgp", bufs=4) as dgp,
            tc.tile_pool(name="pacc", bufs=1, space="PSUM") as paccp,
            tc.tile_pool(name="accp", bufs=2) as accp,
        ):
            # ---- constant loads ----
            wqh_sb, wql_sb, xth_sb, xtl_sb = [], [], [], []
            for k in range(4):
                ksl = slice(k * 128, (k + 1) * 128)
                t = cpool.tile([128, N_HEADS * D_KEYS], BF16, tag=f"wqh{k}")
                nc.sync.dma_start(out=t[:], in_=wqh_d[ksl, :])
                wqh_sb.append(t)
                t = cpool.tile([128, N_HEADS * D_KEYS], BF16, tag=f"wql{k}")
                nc.sync.dma_start(out=t[:], in_=wql_d[ksl, :])
                wql_sb.append(t)
                t = cpool.tile([128, BC], BF16, tag=f"xth{k}")
                nc.sync.dma_start(out=t[:], in_=xth_d[ksl, :])
                xth_sb.append(t)
                t = cpool.tile([128, BC], BF16, tag=f"xtl{k}")
                nc.sync.dma_start(out=t[:], in_=xtl_d[ksl, :])
                xtl_sb.append(t)
            xtok_bf = []
            for tb in range(TB):
                tb16 = cpool.tile([128, D], BF16, tag=f"xtokb{tb}")
                nc.sync.dma_start(
                    out=tb16[:], in_=xtokb_d[tb * 128:(tb + 1) * 128, :])
                xtok_bf.append(tb16)
            kpph_sb = cpool.tile([128, N_HEADS, N_KEYS], BF16, tag="kpph")
            nc.sync.dma_start(out=kpph_sb[:], in_=kpph_d[:, :, :])
            kppl_sb = cpool.tile([128, N_HEADS, N_KEYS], BF16, tag="kppl")
            nc.sync.dma_start(out=kppl_sb[:], in_=kppl_d[:, :, :])
            id01_sb = cpool.tile([128, 128], BF16, tag="id01")
            nc.sync.dma_start(out=id01_sb[:], in_=id01_d[:, :])
            bqp_sb = cpool.tile([128, N_HEADS], F32, tag="bqp")
            nc.sync.dma_start(out=bqp_sb[:], in_=bqp_d[:, :])
            iota8 = cpool.tile([128, 8], U16, tag="iota8")
            nc.gpsimd.iota(iota8[:], pattern=[[1, 8]], base=0, channel_multiplier=0)

            # ---- qT: [feature, token]; feature f = m*128 + p ----
            # fp32-accurate via bf16 hi/lo 3-pass: q = xh*Wh + xh*Wl + xl*Wh
            qh_all = qtpool.tile([128, N_HEADS, BC], BF16, tag="qh_all")
            ql_all = qtpool.tile([128, N_HEADS, BC], BF16, tag="ql_all")
            for m in range(N_HEADS):
                msl = slice(m * 128, (m + 1) * 128)
                ps = psq.tile([128, BC], F32, tag="psq")
                n = 0
                for k in range(4):
                    for lw, rx in (
                        (wqh_sb[k], xth_sb[k]),
                        (wqh_sb[k], xtl_sb[k]),
                        (wql_sb[k], xth_sb[k]),
                    ):
                        nc.tensor.matmul(
                            out=ps[:], lhsT=lw[:, msl], rhs=rx[:],
                            start=(n == 0), stop=(n == 11))
                        n += 1
                tq = scpool.tile([128, BC], F32, tag="tq")
                nc.vector.tensor_scalar(
                    out=tq[:], in0=ps[:],
                    scalar1=bqp_sb[:, m:m + 1], scalar2=None, op0=OP.add,
                )
                nc.vector.tensor_copy(out=qh_all[:, m, :], in_=tq[:])
                nc.vector.tensor_tensor(
                    out=ql_all[:, m, :], in0=tq[:], in1=qh_all[:, m, :],
                    op=OP.subtract)

            idx32 = []
            v8s = []
            for tb in range(TB):
                tsl = slice(tb * 128, (tb + 1) * 128)
                s1t = st2.tile([128, 64], F32, tag=f"s1t{tb}")
                s2t = st2.tile([128, 64], F32, tag=f"s2t{tb}")
                i1 = st2.tile([128, 64], U16, tag=f"i1{tb}")
                i2 = st2.tile([128, 64], U16, tag=f"i2{tb}")
                # ---- scores + stage-1 top8 (exact) ----
                for m in range(N_HEADS):
                    for half, (st_, ix) in enumerate(
                        ((s1t, i1), (s2t, i2))
                    ):
                        hp = slice(half * HALF, (half + 1) * HALF)
                        ps = pss.tile([128, N_KEYS], F32, tag="pss")
                        for n, (lq, rk) in enumerate((
                            (qh_all, kpph_sb),
                            (qh_all, kppl_sb),
                            (ql_all, kpph_sb),
                        )):
                            nc.tensor.matmul(
                                out=ps[:],
                                lhsT=lq[hp, m, tsl],
                                rhs=rk[hp, m, :],
                                start=(n == 0), stop=(n == 2),
                            )
                        s_sb = scpool.tile([128, N_KEYS], F32, tag="s_sb")
                        nc.vector.tensor_copy(out=s_sb[:], in_=ps[:])
                        nc.vector.max(out=st_[:, m * 8:(m + 1) * 8], in_=s_sb[:])
                        nc.vector.max_index(
                            out=ix[:, m * 8:(m + 1) * 8],
                            in_max=st_[:, m * 8:(m + 1) * 8],
                            in_values=s_sb[:],
                        )

                # ---- stage-2: 8x8 combo scores, top8 ----
                cs = st2.tile([128, 512], F32, tag=f"cs{tb}")
                for m in range(N_HEADS):
                    nc.vector.tensor_tensor(
                        out=cs[:, m * 64:(m + 1) * 64].rearrange(
                            "p (a b) -> p a b", a=8),
                        in0=s1t[:, m * 8:(m + 1) * 8].unsqueeze(2).to_broadcast(
                            [128, 8, 8]),
                        in1=s2t[:, m * 8:(m + 1) * 8].unsqueeze(1).to_broadcast(
                            [128, 8, 8]),
                        op=OP.add,
                    )
                v8 = st2.tile([128, 64], F32, tag=f"v8{tb}")
                n8 = st2.tile([128, 64], U16, tag=f"n8{tb}")
                for m in range(N_HEADS):
                    nc.vector.max(out=v8[:, m * 8:(m + 1) * 8],
                                  in_=cs[:, m * 64:(m + 1) * 64])
                    nc.vector.max_index(
                        out=n8[:, m * 8:(m + 1) * 8],
                        in_max=v8[:, m * 8:(m + 1) * 8],
                        in_values=cs[:, m * 64:(m + 1) * 64])
                k1 = st2.tile([128, 64], U16, tag=f"k1{tb}")
                nc.vector.tensor_scalar(
                    out=k1[:], in0=n8[:], scalar1=3, scalar2=None,
                    op0=OP.logical_shift_right)
                k2 = st2.tile([128, 64], U16, tag=f"k2{tb}")
                nc.vector.tensor_scalar(
                    out=k2[:], in0=n8[:], scalar1=7, scalar2=None,
                    op0=OP.bitwise_and)

                # resolve winners' sub-key ids: isel[p,m,j] = i[p,m,k1[p,m,j]]
                sels = []
                for kk, ix in ((k1, i1), (k2, i2)):
                    eq = eqs.tile([128, 512], U16, tag="eq")
                    nc.vector.tensor_tensor(
                        out=eq[:, :].rearrange("p (m j k) -> p m j k", m=8, j=8),
                        in0=kk[:, :].rearrange("p (m j) -> p m j", m=8)
                            .unsqueeze(3).to_broadcast([128, 8, 8, 8]),
                        in1=iota8[:, :].unsqueeze(1).unsqueeze(1)
                            .to_broadcast([128, 8, 8, 8]),
                        op=OP.is_equal)
                    prod = eqs.tile([128, 512], U16, tag="prod")
                    nc.vector.tensor_tensor(
                        out=prod[:, :].rearrange("p (m j k) -> p m j k", m=8, j=8),
                        in0=eq[:, :].rearrange("p (m j k) -> p m j k", m=8, j=8),
                        in1=ix[:, :].rearrange("p (m k) -> p m k", m=8)
                            .unsqueeze(2).to_broadcast([128, 8, 8, 8]),
                        op=OP.mult)
                    sel = st2.tile([128, 64], U16, tag=f"sel{len(sels)}{tb}")
                    with nc.allow_low_precision(
                            reason="one-hot uint16 sum, values <= 255"):
                        nc.vector.reduce_sum(
                            out=sel[:],
                            in_=prod[:, :].rearrange("p (mj k) -> p mj k", k=8),
                            axis=X)
                    sels.append(sel)
                idx16 = st2.tile([128, 64], U16, tag=f"idx16{tb}")
                nc.vector.tensor_scalar(
                    out=idx16[:], in0=sels[0][:], scalar1=256, scalar2=None,
                    op0=OP.mult)
                nc.vector.tensor_tensor(
                    out=idx16[:], in0=idx16[:], in1=sels[1][:], op=OP.add)
                ix32 = st2.tile([128, 64], I32, tag=f"idx32{tb}")
                nc.vector.tensor_copy(out=ix32[:], in_=idx16[:])
                idx32.append(ix32)
                v8s.append(v8)

            if stage == "routing":
                for tb in range(TB):
                    dbg = st2.tile([128, 64], F32, tag=f"dbg{tb}")
                    nc.vector.tensor_copy(out=dbg[:], in_=idx32[tb][:])
                    nc.sync.dma_start(
                        out=out_d[tb * 128:(tb + 1) * 128, 0:64], in_=dbg[:])
                    nc.sync.dma_start(
                        out=out_d[tb * 128:(tb + 1) * 128, 64:128],
                        in_=v8s[tb][:])

            # ---- softmax over each head's top-8 ----
            ws = []
            for tb in (() if stage == "routing" else range(TB)):
                v8 = v8s[tb]
                rmax = st2.tile([128, 8], F32, tag=f"rmax{tb}")
                nc.vector.reduce_max(
                    out=rmax[:], in_=v8[:, :].rearrange("p (m k) -> p m k", m=8),
                    axis=X)
                ex = st2.tile([128, 64], F32, tag=f"ex{tb}")
                nc.vector.tensor_tensor(
                    out=ex[:, :].rearrange("p (m k) -> p m k", m=8),
                    in0=v8[:, :].rearrange("p (m k) -> p m k", m=8),
                    in1=rmax[:, :].unsqueeze(2).to_broadcast([128, 8, 8]),
                    op=OP.subtract)
                nc.scalar.activation(out=ex[:], in_=ex[:], func=ACTF.Exp)
                rsum = st2.tile([128, 8], F32, tag=f"rsum{tb}")
                nc.vector.reduce_sum(
                    out=rsum[:], in_=ex[:, :].rearrange("p (m k) -> p m k", m=8),
                    axis=X)
                rinv = st2.tile([128, 8], F32, tag=f"rinv{tb}")
                nc.vector.reciprocal(out=rinv[:], in_=rsum[:])
                w8 = st2.tile([128, 64], F32, tag=f"w8{tb}")
                nc.vector.tensor_tensor(
                    out=w8[:, :].rearrange("p (m k) -> p m k", m=8),
                    in0=ex[:, :].rearrange("p (m k) -> p m k", m=8),
                    in1=rinv[:, :].unsqueeze(2).to_broadcast([128, 8, 8]),
                    op=OP.mult)
                ws.append(w8)

            # ---- main loop: multi-row gathers, fused inner, combine ----
            tbs = () if stage == "routing" else tuple(range(TB))
            inner = {}
            va = {}
            pacc = {}
            for tb in tbs:
                inner[tb] = st2.tile([128, 64], F32, tag=f"inner{tb}",
                                     name=f"inner{tb}")
                va[tb] = st2.tile([128, 64], BF16, tag=f"va{tb}",
                                  name=f"va{tb}")
                pacc[tb] = [
                    paccp.tile([128, D], F32, tag=f"pacc{tb}{b}",
                               name=f"pacc{tb}{b}")
                    for b in range(2)
                ]
            pages = {}
            for tb in tbs:
                for col in range(64):
                    page = pgp.tile([128, 2 * D], BF16, tag="page",
                                    name=f"pg{tb}_{col}")
                    pages[(tb, col)] = page
                    nc.gpsimd.indirect_dma_start(
                        out=page[:], out_offset=None,
                        in_=wb_d[:, :],
                        in_offset=IndirectOffsetOnAxis(
                            ap=idx32[tb][:, col:col + 1], axis=0),
                    )
            for tb in tbs:
                for g in range(NG):
                    for j in range(GS):
                        col = g * GS + j
                        scr = ttrp.tile([128, D], BF16, tag="ttr_scr")
                        nc.vector.tensor_tensor(
                            out=scr[:], in0=pages[(tb, col)][:, 0:D],
                            in1=xtok_bf[tb][:], op=OP.mult)
                        scr2 = ttrp.tile([128, D], BF16, tag="ttr_scr2")
                        nc.scalar.activation(
                            out=scr2[:], in_=scr[:], func=ACTF.Copy,
                            accum_out=inner[tb][:, col:col + 1])
                    gs = slice(g * GS, (g + 1) * GS)
                    rl8 = st2.tile([128, GS], F32, tag=f"rl{tb}")
                    nc.scalar.activation(
                        out=rl8[:], in_=inner[tb][:, gs], func=ACTF.Relu)
                    nc.vector.tensor_tensor(
                        out=va[tb][:, gs], in0=rl8[:], in1=ws[tb][:, gs],
                        op=OP.mult)
                    diag = dgp.tile([128, GS, 128], BF16, tag="diag")
                    nc.vector.tensor_tensor(
                        out=diag[:],
                        in0=va[tb][:, gs].unsqueeze(2).to_broadcast(
                            [128, GS, 128]),
                        in1=id01_sb[:].unsqueeze(1).to_broadcast(
                            [128, GS, 128]),
                        op=OP.mult)
                    for j in range(GS):
                        col = g * GS + j
                        nc.tensor.matmul(
                            out=pacc[tb][col % 2][:],
                            lhsT=diag[:, j, :],
                            rhs=pages[(tb, col)][:, D:2 * D],
                            start=(col < 2), stop=(col >= 62))
            for tb in tbs:
                acc0 = accp.tile([128, D], F32, tag=f"acc0{tb}")
                nc.scalar.copy(out=acc0[:], in_=pacc[tb][0][:])
                acc_sb = accp.tile([128, D], F32, tag=f"acc{tb}")
                nc.vector.tensor_tensor(
                    out=acc_sb[:], in0=acc0[:], in1=pacc[tb][1][:],
                    op=OP.add)
                nc.sync.dma_start(
                    out=out_d[tb * 128:(tb + 1) * 128, :], in_=acc_sb[:])

    nc.compile()
    return nc


_NC_CACHE = None


def _get_nc():
    global _NC_CACHE
    if _NC_CACHE is None:
        _NC_CACHE = build_nc()
    return _NC_CACHE


def _split_bf16(a):
    import ml_dtypes
    ah = np.ascontiguousarray(a).astype(ml_dtypes.bfloat16)
    al = (a - ah.astype(np.float32)).astype(ml_dtypes.bfloat16)
    return np.ascontiguousarray(ah), np.ascontiguousarray(al)


def _prep_in_maps(inputs):
    q = np.ascontiguousarray(np.asarray(inputs["queries"], dtype=np.float32))
    Wq = np.ascontiguousarray(np.asarray(inputs["Wq"], dtype=np.float32))
    bq = np.asarray(inputs["bq"], dtype=np.float32)
    keys = np.asarray(inputs["keys"], dtype=np.float32)
    wd = np.asarray(inputs["w_down"], dtype=np.float32)
    wu = np.asarray(inputs["w_up"], dtype=np.float32)
    import ml_dtypes
    wb = np.ascontiguousarray(
        np.concatenate([wd, wu], axis=1).astype(ml_dtypes.bfloat16))
    id01 = np.eye(128, dtype=np.float32).astype(ml_dtypes.bfloat16)

    x = q.reshape(B, D)
    wqh, wql = _split_bf16(Wq)
    # bqp[p, m] = bq[m*128 + p]
    bqp = np.ascontiguousarray(bq.reshape(N_HEADS, 128).T)
    # kpp[p, m, n] = keys[m, p//64, n, p%64]
    kp1 = keys[:, 0].transpose(2, 0, 1)   # [64, H, N]
    kp2 = keys[:, 1].transpose(2, 0, 1)
    kpp = np.concatenate([kp1, kp2], axis=0)
    kpph, kppl = _split_bf16(kpp)

    in_maps = []
    for c in range(N_CORES):
        xc = x[c * BC:(c + 1) * BC]
        xth, xtl = _split_bf16(xc.T)
        in_maps.append({
            "xtokb": np.ascontiguousarray(xc).astype(ml_dtypes.bfloat16),
            "xth": xth,
            "xtl": xtl,
            "wqh": wqh,
            "wql": wql,
            "bqp": bqp,
            "kpph": kpph,
            "kppl": kppl,
            "wb": wb,
            "id01": id01,
        })
    return in_maps


def run(inputs, trace=False):
    """Run on 8 NeuronCores; returns (out [2,1024,512], BassKernelResults)."""
    nc = _get_nc()
    in_maps = _prep_in_maps(inputs)
    res = run_bass_kernel_spmd(
        nc, in_maps, core_ids=list(range(N_CORES)), trace=trace)
    out = np.concatenate(
        [res.results[c]["out"] for c in range(N_CORES)], axis=0)
    return out.reshape(2, 1024, D), res


def kernel(**inputs) -> np.ndarray:
    out, _ = run(inputs, trace=False)
    return out


# revision 21
# speedup vs baseline: 1.1186x; 1.1186x over previous
"""PEER / product-key MoE routing kernel for Trainium2 (8 NeuronCores).

Strategy: data-parallel over tokens. Each of the 8 cores gets 256 of the
2048 tokens plus a full replica of the expert tables in its DRAM. Routing
(q projection, product-key scores, two-stage top-8), expert-row gathers,
and the PEER combine all run on-device. No collectives are needed; the
host only slices/packs inputs and concatenates the per-core outputs.

v6: per-head software pipeline around the expert-gather stream (128
indirect DMAs of 128x2KB row-pairs, HBM-random-read bound at ~183 GB/s,
the kernel floor). Head m's routing chain (PE qproj / scores as bf16
hi+lo 3-pass, exact to fp32 selection; DVE top8 straight from score
PSUM; fused one-hot winner resolve; softmax) is emitted per token-block
with its gathers immediately after, and head m-1's consume chain (DVE
bf16 multiply + ACT accum inner products, PE diag(vals) @ w_up PSUM
combine) is emitted behind head m's, so no engine queue ever blocks the
gather stream. Constants arrive in 4 packed DMAs so the first qproj
starts ~6us in. Tables are bf16 (rel err ~4e-3).
"""

import numpy as np

import concourse.mybir as mybir
from concourse import bacc
from concourse.bass import IndirectOffsetOnAxis
from concourse.tile import TileContext
from concourse.bass_utils import run_bass_kernel_spmd

N_CORES = 8
N_HEADS = 8
D_KEYS = 128
HALF = 64
N_KEYS = 256
TOP_K = 8
D = 512
B = 2048           # total tokens
BC = B // N_CORES  # tokens per core (256)
TB = BC // 128     # token blocks per core (2)
F32 = mybir.dt.float32
U16 = mybir.dt.uint16
I32 = mybir.dt.int32
BF16 = mybir.dt.bfloat16
X = mybir.AxisListType.X
OP = mybir.AluOpType
ACTF = mybir.ActivationFunctionType

# blob A: k=0 slices of wqh|xth|wql|xtl -> [128, 2560]
BA = 2560
# blob B: kpph|kppl|id01|xtok0|xtok1 -> [128, 5248]
BB = 2048 + 2048 + 128 + 512 + 512
# blob C: k=1..3 slices of wqh|xth|wql|xtl -> [128, 7680]
BCC = 3 * 2560


def build_nc(stage="full"):
    nc = bacc.Bacc("TRN2", target_bir_lowering=False)

    bqp_d = nc.dram_tensor("bqp", [128, N_HEADS], F32, kind="ExternalInput")
    blobA_d = nc.dram_tensor("blobA", [128, BA], BF16, kind="ExternalInput")
    blobB_d = nc.dram_tensor("blobB", [128, BB], BF16, kind="ExternalInput")
    blobC_d = nc.dram_tensor("blobC", [128, BCC], BF16, kind="ExternalInput")
    wb_d = nc.dram_tensor("wb", [N_KEYS * N_KEYS, 2 * D], BF16,
                          kind="ExternalInput")
    out_d = nc.dram_tensor("out", [BC, D], F32, kind="ExternalOutput")

    tbs = (0,) if stage == "routing" else tuple(range(TB))

    with TileContext(nc) as tc:
        with (
            tc.tile_pool(name="const", bufs=1) as cpool,
            tc.tile_pool(name="qt", bufs=1) as qtpool,
            tc.tile_pool(name="psq", bufs=1, space="PSUM") as psq,
            tc.tile_pool(name="pss", bufs=2, space="PSUM") as pss,
            tc.tile_pool(name="sc", bufs=4) as scpool,
            tc.tile_pool(name="st2", bufs=1) as st2,
            tc.tile_pool(name="eqs", bufs=2) as eqs,
            tc.tile_pool(name="pgp", bufs=34) as pgp,
            tc.tile_pool(name="ttr", bufs=2) as ttrp,
            tc.tile_pool(name="dgp", bufs=4) as dgp,
            tc.tile_pool(name="pacc", bufs=1, space="PSUM") as paccp,
            tc.tile_pool(name="accp", bufs=2) as accp,
        ):
            # ---- constant loads (4 packed DMAs; head-0 deps first) ----
            bqp_sb = cpool.tile([128, N_HEADS], F32, tag="bqp")
            nc.sync.dma_start(out=bqp_sb[:], in_=bqp_d[:, :])
            blobA = cpool.tile([128, BA], BF16, tag="blobA")
            nc.sync.dma_start(out=blobA[:], in_=blobA_d[:, :])
            blobB = cpool.tile([128, BB], BF16, tag="blobB")
            nc.sync.dma_start(out=blobB[:], in_=blobB_d[:, :])
            blobC = cpool.tile([128, BCC], BF16, tag="blobC")
            nc.sync.dma_start(out=blobC[:], in_=blobC_d[:, :])

            def kblob(k):
                """(tile, base) for the k-th 2560-col wqh|xth|wql|xtl group."""
                return (blobA, 0) if k == 0 else (blobC, (k - 1) * 2560)

            def kpp_sl(hi, hp, m):
                base = 0 if hi else 2048
                return blobB[hp, base + m * 256:base + (m + 1) * 256]

            id01_sb = blobB[:, 4096:4224]
            xtok_bf = [blobB[:, 4224 + tb * 512:4224 + (tb + 1) * 512]
                       for tb in range(TB)]
            iota8 = cpool.tile([128, 8], U16, tag="iota8")
            nc.gpsimd.iota(iota8[:], pattern=[[1, 8]], base=0,
                           channel_multiplier=0)

            # ---- persistent per-tb tiles ----
            qh_all = qtpool.tile([128, N_HEADS, BC], BF16, tag="qh_all")
            ql_all = qtpool.tile([128, N_HEADS, BC], BF16, tag="ql_all")
            s1t, s2t, i12t, v8s, wss = {}, {}, {}, {}, {}
            idx32, inner, va, pacc = {}, {}, {}, {}
            for tb in range(TB):
                s1t[tb] = st2.tile([128, 64], F32, tag=f"s1t{tb}",
                                   name=f"s1t{tb}")
                s2t[tb] = st2.tile([128, 64], F32, tag=f"s2t{tb}",
                                   name=f"s2t{tb}")
                i12t[tb] = st2.tile([128, 2, 64], U16, tag=f"i12{tb}",
                                    name=f"i12{tb}")
                v8s[tb] = st2.tile([128, 64], F32, tag=f"v8{tb}",
                                   name=f"v8{tb}")
                wss[tb] = st2.tile([128, 64], F32, tag=f"ws{tb}",
                                   name=f"ws{tb}")
                idx32[tb] = st2.tile([128, 64], I32, tag=f"idx32{tb}",
                                     name=f"idx32{tb}")
                inner[tb] = st2.tile([128, 64], F32, tag=f"inner{tb}",
                                     name=f"inner{tb}")
                va[tb] = st2.tile([128, 64], BF16, tag=f"va{tb}",
                                  name=f"va{tb}")
            for tb in tbs:
                pacc[tb] = [
                    paccp.tile([128, D], F32, tag=f"pacc{tb}{b}",
                               name=f"pacc{tb}{b}")
                    for b in range(2)
                ]
            pages = {}

            def head_chain(m):
                """qproj + per-tb (scores, top8, idx, softmax, gathers)."""
                gs = slice(m * 8, (m + 1) * 8)
                # qproj: bf16 hi/lo 3-pass, fp32 PSUM accumulation
                ps = psq.tile([128, BC], F32, tag="psq")
                n = 0
                for k in range(4):
                    t, base = kblob(k)
                    wh = slice(base + m * 128, base + (m + 1) * 128)
                    wl = slice(base + 1280 + m * 128, base + 1280 + (m + 1) * 128)
                    xh = slice(base + 1024, base + 1280)
                    xl = slice(base + 2304, base + 2560)
                    for lw, rx in ((wh, xh), (wh, xl), (wl, xh)):
                        nc.tensor.matmul(
                            out=ps[:], lhsT=t[:, lw], rhs=t[:, rx],
                            start=(n == 0), stop=(n == 11))
                        n += 1
                tq = scpool.tile([128, BC], F32, tag="tq")
                nc.vector.tensor_scalar(
                    out=tq[:], in0=ps[:],
                    scalar1=bqp_sb[:, m:m + 1], scalar2=None, op0=OP.add)
                nc.vector.tensor_copy(out=qh_all[:, m, :], in_=tq[:])
                nc.vector.tensor_tensor(
                    out=ql_all[:, m, :], in0=tq[:], in1=qh_all[:, m, :],
                    op=OP.subtract)
                for tb in tbs:
                    tsl = slice(tb * 128, (tb + 1) * 128)
                    for half, st_ in enumerate((s1t[tb], s2t[tb])):
                        hp = slice(half * HALF, (half + 1) * HALF)
                        ps2 = pss.tile([128, N_KEYS], F32, tag="pss")
                        for n2, (lq, hi) in enumerate((
                            (qh_all, True),
                            (qh_all, False),
                            (ql_all, True),
                        )):
                            nc.tensor.matmul(
                                out=ps2[:],
                                lhsT=lq[hp, m, tsl],
                                rhs=kpp_sl(hi, hp, m),
                                start=(n2 == 0), stop=(n2 == 2))
                        # stage-1 top-8 straight from PSUM
                        nc.vector.max(out=st_[:, gs], in_=ps2[:])
                        nc.vector.max_index(
                            out=i12t[tb][:, half, gs], in_max=st_[:, gs],
                            in_values=ps2[:])
                    # stage-2: 8x8 combo scores, top8
                    cs = scpool.tile([128, 64], F32, tag="cs")
                    nc.vector.tensor_tensor(
                        out=cs[:].rearrange("p (a b) -> p a b", a=8),
                        in0=s1t[tb][:, gs].unsqueeze(2).to_broadcast(
                            [128, 8, 8]),
                        in1=s2t[tb][:, gs].unsqueeze(1).to_broadcast(
                            [128, 8, 8]),
                        op=OP.add)
                    nc.vector.max(out=v8s[tb][:, gs], in_=cs[:])
                    n8 = scpool.tile([128, 8], U16, tag="n8")
                    nc.vector.max_index(
                        out=n8[:], in_max=v8s[tb][:, gs], in_values=cs[:])
                    # winner positions in each stage-1 list: k1 = n8>>3, k2 = n8&7
                    kk2 = scpool.tile([128, 2, 8], U16, tag="kk2")
                    nc.vector.tensor_scalar(
                        out=kk2[:, 0, :], in0=n8[:], scalar1=3, scalar2=None,
                        op0=OP.logical_shift_right)
                    nc.vector.tensor_scalar(
                        out=kk2[:, 1, :], in0=n8[:], scalar1=7, scalar2=None,
                        op0=OP.bitwise_and)
                    # resolve winners' sub-key ids via one fused one-hot pass
                    eq = eqs.tile([128, 2, 8, 8], U16, tag="eq")
                    nc.vector.tensor_tensor(
                        out=eq[:],
                        in0=kk2[:].unsqueeze(3).to_broadcast([128, 2, 8, 8]),
                        in1=iota8[:].unsqueeze(1).unsqueeze(1).to_broadcast(
                            [128, 2, 8, 8]),
                        op=OP.is_equal)
                    prod = eqs.tile([128, 2, 8, 8], U16, tag="prod")
                    nc.vector.tensor_tensor(
                        out=prod[:],
                        in0=eq[:],
                        in1=i12t[tb][:, :, gs].unsqueeze(2).to_broadcast(
                            [128, 2, 8, 8]),
                        op=OP.mult)
                    sel2 = scpool.tile([128, 2, 8], U16, tag="sel2")
                    with nc.allow_low_precision(
                            reason="one-hot uint16 sum, values <= 255"):
                        nc.vector.reduce_sum(out=sel2[:], in_=prod[:], axis=X)
                    idx16 = scpool.tile([128, 8], U16, tag="idx16")
                    nc.vector.tensor_scalar(
                        out=idx16[:], in0=sel2[:, 0, :], scalar1=256,
                        scalar2=None, op0=OP.mult)
                    nc.vector.tensor_tensor(
                        out=idx32[tb][:, gs], in0=idx16[:],
                        in1=sel2[:, 1, :], op=OP.add)
                    # softmax over this head's top-8
                    rmax = scpool.tile([128, 1], F32, tag="rmax")
                    nc.vector.reduce_max(
                        out=rmax[:], in_=v8s[tb][:, gs].unsqueeze(1), axis=X)
                    ex = scpool.tile([128, 8], F32, tag="ex")
                    nc.vector.tensor_scalar(
                        out=ex[:], in0=v8s[tb][:, gs],
                        scalar1=rmax[:], scalar2=None, op0=OP.subtract)
                    nc.scalar.activation(out=ex[:], in_=ex[:], func=ACTF.Exp)
                    rsum = scpool.tile([128, 1], F32, tag="rsum")
                    nc.vector.reduce_sum(
                        out=rsum[:], in_=ex[:].unsqueeze(1), axis=X)
                    rinv = scpool.tile([128, 1], F32, tag="rinv")
                    nc.vector.reciprocal(out=rinv[:], in_=rsum[:])
                    nc.vector.tensor_scalar(
                        out=wss[tb][:, gs], in0=ex[:],
                        scalar1=rinv[:], scalar2=None, op0=OP.mult)
                    # expert-row gathers for this head/token-block
                    if stage != "routing":
                        for j in range(8):
                            col = m * 8 + j
                            page = pgp.tile([128, 2 * D], BF16, tag="page",
                                            name=f"pg{tb}_{col}")
                            pages[(tb, col)] = page
                            nc.gpsimd.indirect_dma_start(
                                out=page[:], out_offset=None,
                                in_=wb_d[:, :],
                                in_offset=IndirectOffsetOnAxis(
                                    ap=idx32[tb][:, col:col + 1], axis=0),
                            )

            def consume(m):
                sub = 4 if m == N_HEADS - 1 else 8
                for tb in tbs:
                    for j0 in range(0, 8, sub):
                        gs = slice(m * 8 + j0, m * 8 + j0 + sub)
                        for j in range(j0, j0 + sub):
                            col = m * 8 + j
                            scr = ttrp.tile([128, D], BF16, tag="ttr_scr")
                            nc.vector.tensor_tensor(
                                out=scr[:], in0=pages[(tb, col)][:, 0:D],
                                in1=xtok_bf[tb], op=OP.mult)
                            if m == N_HEADS - 1:
                                nc.vector.reduce_sum(
                                    out=inner[tb][:, col:col + 1],
                                    in_=scr[:].unsqueeze(1), axis=X)
                            else:
                                scr2 = ttrp.tile([128, D], BF16,
                                                 tag="ttr_scr2")
                                nc.scalar.activation(
                                    out=scr2[:], in_=scr[:], func=ACTF.Copy,
                                    accum_out=inner[tb][:, col:col + 1])
                        rl8 = st2.tile([128, sub], F32, tag=f"rl{tb}{j0}")
                        nc.scalar.activation(
                            out=rl8[:], in_=inner[tb][:, gs], func=ACTF.Relu)
                        nc.vector.tensor_tensor(
                            out=va[tb][:, gs], in0=rl8[:], in1=wss[tb][:, gs],
                            op=OP.mult)
                        diag = dgp.tile([128, sub, 128], BF16, tag="diag",
                                        name=f"diag{tb}")
                        nc.vector.tensor_tensor(
                            out=diag[:],
                            in0=va[tb][:, gs].unsqueeze(2).to_broadcast(
                                [128, sub, 128]),
                            in1=id01_sb.unsqueeze(1).to_broadcast(
                                [128, sub, 128]),
                            op=OP.mult)
                        for j in range(j0, j0 + sub):
                            col = m * 8 + j
                            nc.tensor.matmul(
                                out=pacc[tb][col % 2][:],
                                lhsT=diag[:, j - j0, :],
                                rhs=pages[(tb, col)][:, D:2 * D],
                                start=(col < 2), stop=(col >= 62))

            # ---- software-pipelined per-head chains ----
            if stage == "routing":
                for m in range(N_HEADS):
                    head_chain(m)
                for tb in tbs:
                    dbg = st2.tile([128, 64], F32, tag=f"dbg{tb}")
                    nc.vector.tensor_copy(out=dbg[:], in_=idx32[tb][:])
                    nc.sync.dma_start(
                        out=out_d[tb * 128:(tb + 1) * 128, 0:64], in_=dbg[:])
                    nc.sync.dma_start(
                        out=out_d[tb * 128:(tb + 1) * 128, 64:128],
                        in_=v8s[tb][:])
            else:
                for m in range(N_HEADS):
                    head_chain(m)
                    if m >= 1:
                        consume(m - 1)
                consume(N_HEADS - 1)
                for tb in tbs:
                    acc0 = accp.tile([128, D], F32, tag=f"acc0{tb}")
                    nc.scalar.copy(out=acc0[:], in_=pacc[tb][0][:])
                    acc_sb = accp.tile([128, D], F32, tag=f"acc{tb}")
                    nc.vector.tensor_tensor(
                        out=acc_sb[:], in0=acc0[:], in1=pacc[tb][1][:],
                        op=OP.add)
                    nc.sync.dma_start(
                        out=out_d[tb * 128:(tb + 1) * 128, :], in_=acc_sb[:])

    nc.compile()
    return nc


_NC_CACHE = None


def _get_nc():
    global _NC_CACHE
    if _NC_CACHE is None:
        _NC_CACHE = build_nc()
    return _NC_CACHE


def _split_bf16(a):
    import ml_dtypes
    ah = np.ascontiguousarray(a).astype(ml_dtypes.bfloat16)
    al = (a - ah.astype(np.float32)).astype(ml_dtypes.bfloat16)
    return np.ascontiguousarray(ah), np.ascontiguousarray(al)


def _prep_in_maps(inputs):
    import ml_dtypes
    q = np.ascontiguousarray(np.asarray(inputs["queries"], dtype=np.float32))
    Wq = np.ascontiguousarray(np.asarray(inputs["Wq"], dtype=np.float32))
    bq = np.asarray(inputs["bq"], dtype=np.float32)
    keys = np.asarray(inputs["keys"], dtype=np.float32)
    wd = np.asarray(inputs["w_down"], dtype=np.float32)
    wu = np.asarray(inputs["w_up"], dtype=np.float32)
    wb = np.ascontiguousarray(
        np.concatenate([wd, wu], axis=1).astype(ml_dtypes.bfloat16))
    id01 = np.eye(128, dtype=np.float32).astype(ml_dtypes.bfloat16)

    x = q.reshape(B, D)
    wqh, wql = _split_bf16(Wq)
    # bqp[p, m] = bq[m*128 + p]
    bqp = np.ascontiguousarray(bq.reshape(N_HEADS, 128).T)
    # kpp[p, m, n] = keys[m, p//64, n, p%64]
    kp1 = keys[:, 0].transpose(2, 0, 1)   # [64, H, N]
    kp2 = keys[:, 1].transpose(2, 0, 1)
    kpp = np.concatenate([kp1, kp2], axis=0)
    kpph, kppl = _split_bf16(kpp)
    kpph2 = kpph.reshape(128, 2048)
    kppl2 = kppl.reshape(128, 2048)

    in_maps = []
    for c in range(N_CORES):
        xc = x[c * BC:(c + 1) * BC]
        xth, xtl = _split_bf16(xc.T)
        xtokb = np.ascontiguousarray(xc).astype(ml_dtypes.bfloat16)
        blobA = np.hstack([wqh[0:128], xth[0:128], wql[0:128], xtl[0:128]])
        blobB = np.hstack([kpph2, kppl2, id01,
                           xtokb[0:128], xtokb[128:256]])
        blobC = np.hstack(sum((
            [wqh[k * 128:(k + 1) * 128], xth[k * 128:(k + 1) * 128],
             wql[k * 128:(k + 1) * 128], xtl[k * 128:(k + 1) * 128]]
            for k in range(1, 4)), []))
        in_maps.append({
            "bqp": bqp,
            "blobA": np.ascontiguousarray(blobA),
            "blobB": np.ascontiguousarray(blobB),
            "blobC": np.ascontiguousarray(blobC),
            "wb": wb,
        })
    return in_maps


def run(inputs, trace=False):
    """Run on 8 NeuronCores; returns (out [2,1024,512], BassKernelResults)."""
    nc = _get_nc()
    in_maps = _prep_in_maps(inputs)
    res = run_bass_kernel_spmd(
        nc, in_maps, core_ids=list(range(N_CORES)), trace=trace)
    out = np.concatenate(
        [res.results[c]["out"] for c in range(N_CORES)], axis=0)
    return out.reshape(2, 1024, D), res


def kernel(**inputs) -> np.ndarray:
    out, _ = run(inputs, trace=False)
    return out
